# revision 11
# baseline (speedup 1.0000x reference)
"""Fused LoRA-QKV projection kernel for 8 Trainium2 NeuronCores.

Computes  out = x @ W.T + b + scaling * concat_k((x @ A[k].T) @ B[k].T)
with x:[4,2048,4096] f32, W:[12288,4096], b:[12288], A:[3,16,4096],
B:[3,4096,16]  ->  out:[4,2048,12288] f32.

Sharding (Megatron column-parallel): the out_dim (12288) axis is split
across 8 cores, keeping each of the 3 q/k/v chunks evenly split, so core c
owns rows {k*4096 + c*512 .. k*4096 + (c+1)*512} of W/b (and rows
{c*512..(c+1)*512} of each B[k]).  x and A are replicated.

Per-core compute (bf16 operands, fp32 PSUM accumulation):
  - x tiles are cast f32->bf16 during the SWDGE DMA load, then transposed
    on the TensorEngine (transpose-mode matmul vs identity) to put the
    contraction dim (d) on partitions.
  - W.T is built once in SBUF (bf16, 96KB/partition) the same way.
  - base matmul: out[128 tok, 512 feat] tiles accumulate over 32 d-tiles.
  - LoRA: lowT[3*32pad, 128 tok] = (A_all.T).T @ xT in one MM chain per
    token block (adapters padded to 32 partitions so everything stays
    partition-aligned), scaled+cast to bf16, then a K=16 matmul per
    feature slice accumulates the up-projection into the same PSUM tile.
  - bias is added during the PSUM->SBUF eviction (DVE tensor_add against a
    broadcast bias tile).
"""

import numpy as np

import concourse.bass as bass
import concourse.mybir as mybir
from concourse import bacc
from concourse.masks import make_identity
from concourse.tile import TileContext

IN_DIM = 4096
OUT_DIM = 12288
R = 16
SCALING = 32.0 / R
N_CORES = 8
TOKENS = 4 * 2048
FEATS = OUT_DIM // N_CORES          # 1536 per core
N_SLICE = 512                       # psum tile free size (one bank of fp32)
F_SLICES = FEATS // N_SLICE         # 3 (== adapter count; slice f <-> adapter f)
D_TILES = IN_DIM // 128             # 32
BF = mybir.dt.bfloat16
F32 = mybir.dt.float32


def build_nc(tokens=TOKENS):
    t_blocks = tokens // 128
    nc = bacc.Bacc()
    x = nc.declare_dram_parameter("x", [tokens, IN_DIM], F32, isOutput=False)
    w = nc.declare_dram_parameter("w", [FEATS, IN_DIM], F32, isOutput=False)
    bvec = nc.declare_dram_parameter("bvec", [FEATS], F32, isOutput=False)
    amat = nc.declare_dram_parameter("amat", [3 * R, IN_DIM], F32, isOutput=False)
    bmat = nc.declare_dram_parameter("bmat", [FEATS, R], F32, isOutput=False)
    out = nc.declare_dram_parameter("out", [tokens, FEATS], F32, isOutput=True)

    with TileContext(nc) as tc:
        with (
            tc.tile_pool(name="const", bufs=1) as const,
            tc.tile_pool(name="tpsum", bufs=2, space="PSUM") as tpsum,
            tc.tile_pool(name="bpsum", bufs=3, space="PSUM") as bpsum,
            tc.tile_pool(name="lpsum", bufs=2, space="PSUM") as lpsum,
        ):
            ident = const.tile([128, 128], BF, name="ident")
            make_identity(nc, ident)
            # W.T resident: wt[:, j*FEATS + f] = W[f, j*128 + p]
            wt = const.tile([128, D_TILES * FEATS], BF, name="wt")
            # A.T resident, adapters padded to 32 rows of the M dim:
            # at[:, j*96 + 32*k + r] = A[k, r, j*128 + p]
            at = const.tile([128, D_TILES * 96], BF, name="at")
            # B.T resident: bt[32*k + r, floc] = B_shard[floc, r]
            bt = const.tile([96, FEATS], BF, name="bt")
            # bias broadcast across partitions
            bb = const.tile([128, FEATS], F32, name="bb")
            bap = bvec[:]
            bias_bcast = bass.AP(
                tensor=bap.tensor, offset=bap.offset,
                ap=[[0, 128]] + [list(d) for d in bap.ap],
            )
            nc.sync.dma_start(out=bb, in_=bias_bcast)
            nc.gpsimd.memset(at, 0.0)

            with tc.tile_pool(name="stage", bufs=2) as stage:
                # ---- W prep: cast-load natural rows, PE-transpose to wt ----
                for i in range(FEATS // 128):
                    wst = stage.tile([128, IN_DIM], BF, name="wst")
                    nc.gpsimd.dma_start(out=wst, in_=w[i * 128:(i + 1) * 128, :])
                    for j in range(D_TILES):
                        tp = tpsum.tile([128, 128], BF, name="tp", tag="tp")
                        nc.tensor.transpose(tp, wst[:, j * 128:(j + 1) * 128], ident)
                        nc.vector.tensor_copy(
                            wt[:, j * FEATS + i * 128: j * FEATS + (i + 1) * 128], tp
                        )
                # ---- A prep ----
                ast = stage.tile([3 * R, IN_DIM], BF, name="ast")
                nc.gpsimd.dma_start(out=ast, in_=amat[:, :])
                for j in range(D_TILES):
                    tpa = tpsum.tile([128, 3 * R], BF, name="tpa", tag="tp")
                    nc.tensor.transpose(
                        tpa, ast[:, j * 128:(j + 1) * 128], ident[0:3 * R, 0:3 * R]
                    )
                    for k in range(3):
                        nc.vector.tensor_copy(
                            at[:, j * 96 + 32 * k: j * 96 + 32 * k + R],
                            tpa[:, k * R:(k + 1) * R],
                        )
                # ---- B prep: chunk c of 128 feats belongs to adapter c//4 ----
                for c in range(FEATS // 128):
                    k = c // (N_SLICE // 128)
                    bst = stage.tile([128, R], BF, name="bst")
                    nc.gpsimd.dma_start(out=bst, in_=bmat[c * 128:(c + 1) * 128, :])
                    tpb = tpsum.tile([96, 128], BF, name="tpb", tag="tp")
                    nc.tensor.matmul(
                        tpb[32 * k: 32 * k + R, :], bst, ident,
                        is_transpose=True, tile_position=(0, 32 * k),
                    )
                    nc.vector.tensor_copy(
                        bt[32 * k: 32 * k + R, c * 128:(c + 1) * 128],
                        tpb[32 * k: 32 * k + R, :],
                    )

            with (
                tc.tile_pool(name="xin", bufs=3) as xin_pool,
                tc.tile_pool(name="xt", bufs=2) as xt_pool,
                tc.tile_pool(name="osb", bufs=3) as osb_pool,
                tc.tile_pool(name="lowsb", bufs=2) as lowsb_pool,
            ):
                for t in range(t_blocks):
                    xin = xin_pool.tile([128, IN_DIM], BF, name="xin")
                    nc.gpsimd.dma_start(out=xin, in_=x[t * 128:(t + 1) * 128, :])
                    xt = xt_pool.tile([128, IN_DIM], BF, name="xt")
                    for j in range(D_TILES):
                        tp = tpsum.tile([128, 128], BF, name="tpx", tag="tp")
                        nc.tensor.transpose(tp, xin[:, j * 128:(j + 1) * 128], ident)
                        nc.vector.tensor_copy(xt[:, j * 128:(j + 1) * 128], tp)
                    # lowT[32k + r, tok] = sum_d A[k,r,d] * x[tok,d]
                    lp = lpsum.tile([96, 128], F32, name="lp")
                    for j in range(D_TILES):
                        nc.tensor.matmul(
                            lp, at[:, j * 96:(j + 1) * 96],
                            xt[:, j * 128:(j + 1) * 128],
                            start=(j == 0), stop=(j == D_TILES - 1),
                        )
                    lowt = lowsb_pool.tile([96, 128], BF, name="lowt")
                    nc.scalar.mul(lowt, lp, SCALING)
                    osb = osb_pool.tile([128, FEATS], F32, name="osb")
                    for f in range(F_SLICES):
                        bp = bpsum.tile([128, N_SLICE], F32, name="bp")
                        for j in range(D_TILES):
                            nc.tensor.matmul(
                                bp, xt[:, j * 128:(j + 1) * 128],
                                wt[:, j * FEATS + f * N_SLICE: j * FEATS + (f + 1) * N_SLICE],
                                start=(j == 0), stop=False,
                            )
                        nc.tensor.matmul(
                            bp, lowt[32 * f: 32 * f + R, :],
                            bt[32 * f: 32 * f + R, f * N_SLICE:(f + 1) * N_SLICE],
                            start=False, stop=True,
                        )
                        nc.vector.tensor_add(
                            osb[:, f * N_SLICE:(f + 1) * N_SLICE], bp,
                            bb[:, f * N_SLICE:(f + 1) * N_SLICE],
                        )
                    nc.sync.dma_start(out=out[t * 128:(t + 1) * 128, :], in_=osb)
    nc.compile()
    return nc


def shard_inputs(inputs, tokens=TOKENS):
    """Full inputs -> per-core in_maps (column-parallel on out_dim)."""
    x = np.ascontiguousarray(np.asarray(inputs["x"], dtype=np.float32)).reshape(
        tokens, IN_DIM
    )
    W = np.asarray(inputs["W"], dtype=np.float32).reshape(3, OUT_DIM // 3, IN_DIM)
    b = np.asarray(inputs["b"], dtype=np.float32).reshape(3, OUT_DIM // 3)
    A = np.asarray(inputs["A"], dtype=np.float32).reshape(3 * R, IN_DIM)
    B = np.asarray(inputs["B"], dtype=np.float32)  # [3, 4096, 16]
    in_maps = []
    for c in range(N_CORES):
        sl = slice(c * N_SLICE, (c + 1) * N_SLICE)
        in_maps.append({
            "x": x,
            "w": np.ascontiguousarray(W[:, sl, :]).reshape(FEATS, IN_DIM),
            "bvec": np.ascontiguousarray(b[:, sl]).reshape(FEATS),
            "amat": np.ascontiguousarray(A),
            "bmat": np.ascontiguousarray(B[:, sl, :]).reshape(FEATS, R),
        })
    return in_maps


def unshard_output(results, tokens=TOKENS):
    """Per-core [tokens, 1536] slices -> full [4, 2048, 12288]."""
    full = np.empty((tokens, 3, N_CORES, N_SLICE), dtype=np.float32)
    for c, res in enumerate(results):
        full[:, :, c, :] = res["out"].reshape(tokens, 3, N_SLICE)
    return full.reshape(4, 2048, OUT_DIM)


def run(inputs, tokens=TOKENS, **kwargs):
    from concourse.bass_utils import run_bass_kernel_spmd

    nc = build_nc(tokens)
    in_maps = shard_inputs(inputs, tokens)
    res = run_bass_kernel_spmd(
        nc, in_maps, core_ids=list(range(N_CORES)), **kwargs
    )
    return unshard_output(res.results, tokens), res


class Executor:
    """Compiled 8-core executor mirroring bass2jax.run_bass_via_pjrt, but
    with the jitted callable and device-resident inputs cached so repeated
    executions can be timed without host<->device transfer or retrace."""

    def __init__(self, tokens=TOKENS):
        import jax
        import numpy as _np
        from jax.sharding import Mesh, NamedSharding, PartitionSpec
        from jax.experimental.shard_map import shard_map
        from concourse import bass2jax, mybir as _mybir

        bass2jax.install_neuronx_cc_hook()
        self.jax = jax
        self.tokens = tokens
        nc = build_nc(tokens)
        self.nc = nc

        partition_name = (
            nc.partition_id_tensor.name if nc.partition_id_tensor else None
        )
        in_names, out_names, out_avals, zero_shapes = [], [], [], []
        for alloc in nc.m.functions[0].allocations:
            if not isinstance(alloc, _mybir.MemoryLocationSet):
                continue
            name = alloc.memorylocations[0].name
            if alloc.kind == "ExternalInput":
                if name != partition_name:
                    in_names.append(name)
            elif alloc.kind == "ExternalOutput":
                shape = tuple(alloc.tensor_shape)
                dtype = _mybir.dt.np(alloc.dtype)
                out_names.append(name)
                out_avals.append(jax.core.ShapedArray(shape, dtype))
                zero_shapes.append((shape, dtype))
        n_params = len(in_names)
        n_outs = len(out_names)
        all_names = list(in_names) + list(out_names)
        if partition_name is not None:
            all_names.append(partition_name)
        donate = tuple(range(n_params, n_params + n_outs))

        def _body(*args):
            operands = list(args)
            if partition_name is not None:
                operands.append(bass2jax.partition_id_tensor())
            outs = bass2jax._bass_exec_p.bind(
                *operands,
                out_avals=tuple(out_avals),
                in_names=tuple(all_names),
                out_names=tuple(out_names),
                lowering_input_output_aliases=(),
                sim_require_finite=True,
                sim_require_nnan=True,
                nc=nc,
            )
            return tuple(outs)

        devices = jax.devices()[:N_CORES]
        mesh = Mesh(_np.asarray(devices), ("core",))
        self.mesh = mesh
        self.sharding = NamedSharding(mesh, PartitionSpec("core"))
        in_specs = (PartitionSpec("core"),) * (n_params + n_outs)
        out_specs = (PartitionSpec("core"),) * n_outs
        self.fn = jax.jit(
            shard_map(
                _body, mesh=mesh, in_specs=in_specs,
                out_specs=out_specs, check_rep=False,
            ),
            donate_argnums=donate,
            keep_unused=True,
        )
        self.in_names = in_names
        self.out_names = out_names
        self.out_avals = out_avals
        self.zero_shapes = zero_shapes

    def place_inputs(self, inputs):
        import numpy as _np
        in_maps = shard_inputs(inputs, self.tokens)
        concat = [
            _np.concatenate([m[name] for m in in_maps], axis=0)
            for name in self.in_names
        ]
        return [self.jax.device_put(a, self.sharding) for a in concat]

    def make_zeros(self):
        import numpy as _np
        return [
            self.jax.device_put(
                _np.zeros((N_CORES * s[0], *s[1:]), d), self.sharding
            )
            for s, d in self.zero_shapes
        ]

    def execute(self, dev_inputs, dev_zeros):
        outs = self.fn(*dev_inputs, *dev_zeros)
        self.jax.block_until_ready(outs)
        return outs

    def to_numpy_output(self, outs):
        import numpy as _np
        full = _np.asarray(outs[0]).reshape(N_CORES, self.tokens, FEATS)
        return unshard_output(
            [{"out": full[c]} for c in range(N_CORES)], self.tokens
        )


def kernel(**inputs) -> np.ndarray:
    out, _ = run(inputs)
    return out
